# revision 17
# baseline (speedup 1.0000x reference)
"""Multi-head attention (with the reference's double-split_proj quirk) on 8
Trainium2 NeuronCores via Bass/Tile — collective-free sequence-parallel
sharding with packed inputs.

Sharding: core c handles batch b = c//4 and row block j in
[512*c', 512*(c'+1)) with c' = c%4, computing all 16 heads for its rows.
Everything the core needs is produced locally:

- The double _split_proj means head i at attention row j reads
  q_proj[b, 16*(j%128) + i, 64*(j//128) + d]. For a 512-row block the
  channel group h' = j//128 spans exactly [4c', 4c'+4) — a 256-wide channel
  slice of Wq — while the source rows s = 16*jj + i cover ALL of q. The
  host supplies q^T with columns permuted to c = 128*i + (s//16) so the
  projected tile splits into per-head contiguous blocks.
- K and V projections are computed for all 16 heads (full Wk/Wv) — 4x
  redundant across the 4 cores of a batch, the price of zero cross-core
  traffic (an AllGather variant measured no faster under this runtime).
- Scores run transposed (k on partitions, local j on free) in bf16; the
  softmax denominator comes from a ones-column appended per head on V; the
  mask is a post-exp bf16 multiply; 1/Z is broadcast across partitions with
  a ones-rank-1 matmul instead of a DRAM round trip.
- The output projection consumes only local features and is written as
  bf16 (host upcasts), halving per-dispatch output traffic.

Dispatch-cost note: under the axon/PJRT runtime each additional IO buffer
costs ~40us PER DISPATCH in the steady-state pipeline (measured: a no-op
kernel with 14 inputs is ~0.4-0.8ms/iter slower than with 1). ALL inputs
are therefore packed host-side into ONE [5251, 2048] bf16 tensor (f32
biases ride along as bf16 and are converted on device; ones rows are
memset constants), and the implicit partition_id input is disabled —
each dispatch carries exactly 2 buffers (pack in, bf16 output out).
On-device tiles address into the pack via strided DMA (rearrange), so
the kernel math is unchanged.
"""

import sys

for _p in ("/opt/trn_rl_repo",):
    if _p not in sys.path:
        sys.path.append(_p)

import numpy as np
import ml_dtypes

import concourse.bass as bass
import concourse.bacc as bacc
import concourse.mybir as mybir
import concourse.tile as tile
from concourse.bass_utils import run_bass_kernel_spmd

B = 2
D = 1024
H = 16
DH = 64
NCORES = 8
S_FULL = 2048

f32 = mybir.dt.float32
f32r = mybir.dt.float32r
bf16 = mybir.dt.bfloat16

_MODULES = {}

# packed-row offsets (2048-wide bf16 rows, S = 2048)
#   qT    [1024, 2048] -> rows [0, 1024)
#   kT    [1024, 2048] -> rows [1024, 2048)
#   vT    [1024, 2048] -> rows [2048, 3072)
#   maskb [2048, 512]  -> rows [3072, 3584)
#   wqT   [1024, 256]  -> rows [3584, 3712)
#   wkT   [1024, 1024] -> rows [3712, 4224)
#   wvT   [1024, 1024] -> rows [4224, 4736)
#   woT   [1024, 1024] -> rows [4736, 5248)
R_QT, R_KT, R_VT = 0, 1024, 2048
R_MB, R_WQ, R_WK, R_WV, R_WO = 3072, 3584, 3712, 4224, 4736
R_BIAS, R_BV = 5248, 5250          # biases padded [128, 32]; bv [1, 1024]
R_TOT = 5251


def build_module(S, stub_collective=False):
    """Build + compile the per-core Bass module (same program on all cores).

    stub_collective is accepted for harness compatibility; there are no
    collectives in this kernel.
    """
    JB = S // 4           # local attention-row block (512 for S=2048)
    KC = S // 128         # 128-wide key/position chunks

    # no collectives and no partition-dependent code: drop the implicit
    # partition_id input (every IO buffer costs ~40us per dispatch)
    nc = bacc.Bacc("TRN2", target_bir_lowering=False, debug=False,
                   num_devices=NCORES, enable_partition_id=False)

    packb_d = nc.dram_tensor("packb", [R_TOT, 2048], bf16,
                             kind="ExternalInput")

    out_d = nc.dram_tensor("ofinal", [D, JB], bf16, kind="ExternalOutput")

    Exp = mybir.ActivationFunctionType.Exp
    Ident = mybir.ActivationFunctionType.Identity

    def eng(i):
        return nc.sync if i % 2 == 0 else nc.gpsimd

    def packed(row0, nrows, cols):
        """[128, cols] tile view of packed rows [row0, row0 + nrows)."""
        apv = packb_d[row0:row0 + nrows, :]
        if cols == 2048:
            return apv
        return apv.rearrange("a (p c) -> (a p) c", c=cols)

    with tile.TileContext(nc) as tc:
        with (
            tc.tile_pool(name="persist", bufs=1) as pp,
            tc.tile_pool(name="stream", bufs=1) as sp,
            tc.tile_pool(name="psS", bufs=1, space="PSUM") as psS,
            tc.tile_pool(name="psP", bufs=1, space="PSUM") as psP,
        ):
            def Stile(name):
                return psS.tile([128, 1024], f32, tag="S", bufs=2, name=name)

            # ---------------- constants + resident weights ----------------
            # biases/bv ride in the pack as bf16 (cast to f32 on device);
            # ones rows are memset constants, no input needed. Tiny loads go
            # on the scalar queue (ACT idle early) so they don't
            # head-of-line-block the critical wq/qT loads.
            bias_bf = pp.tile([128, 32], bf16, tag="biasbf")
            nc.scalar.dma_start(bias_bf[:], packed(R_BIAS, 2, 32))
            bias_sb = pp.tile([128, 32], f32, tag="bias")
            nc.vector.tensor_copy(bias_sb[:], bias_bf[:])
            bvrow_sb = pp.tile([1, D], bf16, tag="bvrow")
            nc.scalar.dma_start(bvrow_sb[:], packb_d[R_BV:R_BV + 1, 0:D])
            onesrow_sb = pp.tile([1, 128], bf16, tag="ones_bf")
            nc.vector.memset(onesrow_sb[:], 1.0)
            # memset can't emit f32r directly; stage via an f32 memset
            ones64_f = pp.tile([1, 64], f32, tag="ones64f")
            nc.vector.memset(ones64_f[:], 1.0)
            ones64_sb = pp.tile([1, 64], f32r, tag="ones64")
            with nc.allow_low_precision(
                    reason="f32r output is bit-identical to f32"):
                nc.vector.tensor_copy(ones64_sb[:], ones64_f[:])
            bq_sb = bias_sb[:, 0:2]

            def bk_col(p8):
                return bias_sb[:, 2 + p8:3 + p8]

            def bo_col(p8):
                return bias_sb[:, 10 + p8:11 + p8]

            # wq resident (critical path at start)
            wq_sb = []
            for ci in range(8):
                t = pp.tile([128, 256], bf16, tag="wq", bufs=8,
                            name=f"wq{ci}")
                eng(ci).dma_start(t[:], packed(R_WQ + 16 * ci, 16, 256))
                wq_sb.append(t)

            # q/k/v^T streamed in halves of the free dim
            def mv_load(row0, ci, h, name):
                t = sp.tile([128, 1024], bf16, tag="mv", bufs=12, name=name)
                eng(ci).dma_start(
                    t[:], packb_d[row0 + 128 * ci:row0 + 128 * (ci + 1),
                                  1024 * h:1024 * (h + 1)])
                return t

            # per-head Q^T / K^T packed in pairs: head i on partitions
            # 64*(i%2) .. +64 of pair tile i//2
            QTp = [pp.tile([128, JB], bf16, tag=f"QTp{p}", name=f"QTp{p}")
                   for p in range(8)]
            KTp = [pp.tile([128, S], bf16, tag=f"KTp{p}", name=f"KTp{p}")
                   for p in range(8)]

            def QTs(i):
                return QTp[i // 2][64 * (i % 2):64 * (i % 2) + 64, :]

            def KTs(i):
                return KTp[i // 2][64 * (i % 2):64 * (i % 2) + 64, :]

            # V+bias augmented with a ones column per head (16 heads x 65)
            VA = [pp.tile([128, 16 * 65], bf16, tag=f"VA{sc}", name=f"VA{sc}")
                  for sc in range(KC)]
            for sc in range(KC):
                nc.vector.memset(
                    VA[sc].rearrange("p (h x) -> p h x", h=16)[:, :, 64:65],
                    1.0)

            # ---------------- Q projection ----------------
            # q_projT[ch_loc, c] for ch_loc in [0,256), c = 128*i + jj.
            # Two free-dim halves; sequential psums so the second chunk's
            # matmuls overlap the first chunk's DVE drain.
            for h in range(2):
                qts = [mv_load(R_QT, ci, h, f"qts{ci}_{h}") for ci in range(8)]
                for p in range(2):
                    pst = Stile(f"qps{p}_{h}")
                    for ci in range(8):
                        for nb in range(2):
                            nc.tensor.matmul(
                                pst[:, 512 * nb:512 * (nb + 1)],
                                wq_sb[ci][:, 128 * p:128 * (p + 1)],
                                qts[ci][:, 512 * nb:512 * (nb + 1)],
                                start=(ci == 0), stop=(ci == 7))
                    # psum free col f (+1024h) = 128*i + jj; partitions
                    # 128p+64*half+d -> head i, hl = 2p+half
                    for half in range(2):
                        hl = 2 * p + half
                        for il in range(8):
                            i = 8 * h + il
                            nc.vector.tensor_scalar_add(
                                QTs(i)[:, 128 * hl:128 * (hl + 1)],
                                pst[64 * half:64 * half + 64,
                                    128 * il:128 * (il + 1)],
                                bq_sb[64 * half:64 * half + 64, p:p + 1],
                            )

            # ---------------- K projection ----------------
            # k_projT[ch, kpos] for all 1024 channels; psum tile per
            # (p8, kpos-half) holds 2 x 512 kpos slices.
            wk_sb = []
            for ci in range(8):
                t = pp.tile([128, D], bf16, tag="wk", bufs=8,
                            name=f"wk{ci}")
                eng(ci + 1).dma_start(t[:], packed(R_WK + 64 * ci, 64, 1024))
                wk_sb.append(t)
            for h in range(2):
                kts = [mv_load(R_KT, ci, h, f"kts{ci}_{h}") for ci in range(8)]
                for p8 in range(8):
                    ps = Stile(f"kps{p8}_{h}")
                    for ci in range(8):
                        for nb in range(2):
                            nc.tensor.matmul(
                                ps[:, 512 * nb:512 * (nb + 1)],
                                wk_sb[ci][:, 128 * p8:128 * (p8 + 1)],
                                kts[ci][:, 512 * nb:512 * (nb + 1)],
                                start=(ci == 0), stop=(ci == 7))
                    # both head-halves of this psum land in contiguous
                    # rows of KTp[p8] with the same bias column: one
                    # full-height add (half the DVE ops at equal cost/op)
                    nc.vector.tensor_scalar_add(
                        KTp[p8][:, 1024 * h:1024 * (h + 1)],
                        ps[:],
                        bk_col(p8),
                    )

            # ---------------- V projection ----------------
            # natural layout: psum [128 s-positions, 1024 ch] per s-chunk,
            # + bv via rank-1 matmul, copied into VA (65-stride per head).
            wv_sb = []
            for ci in range(8):
                t = pp.tile([128, D], bf16, tag="wv", bufs=8,
                            name=f"wv{ci}")
                eng(ci).dma_start(t[:], packed(R_WV + 64 * ci, 64, 1024))
                wv_sb.append(t)
            for h in range(2):
                vts = [mv_load(R_VT, ci, h, f"vts{ci}_{h}") for ci in range(8)]
                for scl in range(8):
                    sc = 8 * h + scl
                    vps = Stile(f"vps{sc}")
                    for ci in range(8):
                        for nb in range(2):
                            nc.tensor.matmul(
                                vps[:, 512 * nb:512 * (nb + 1)],
                                vts[ci][:, 128 * scl:128 * (scl + 1)],
                                wv_sb[ci][:, 512 * nb:512 * (nb + 1)],
                                start=(ci == 0), stop=False)
                    for nb in range(2):
                        nc.tensor.matmul(vps[:, 512 * nb:512 * (nb + 1)],
                                         onesrow_sb[:],
                                         bvrow_sb[:, 512 * nb:512 * (nb + 1)],
                                         start=False, stop=True)
                    nc.vector.tensor_copy(
                        VA[sc].rearrange("p (h x) -> p h x", h=16)[:, :, 0:64],
                        vps.rearrange("p (h d) -> p h d", h=16),
                    )

            # (1 - mask)^T column block, resident in bf16
            maskb_sb = []
            for kc in range(KC):
                t = pp.tile([128, JB], bf16, tag=f"mb{kc}", name=f"mb{kc}")
                eng(kc).dma_start(t[:], packed(R_MB + 32 * kc, 32, 512))
                maskb_sb.append(t)

            # ---------------- attention (all 16 heads, local j rows) -------
            # feature pairs for the output projection: channel chunk ci=hp
            FP = [pp.tile([128, JB], bf16, tag=f"FP{p}", name=f"FP{p}")
                  for p in range(8)]
            for hp in range(8):
                h0, h1 = 2 * hp, 2 * hp + 1
                PV0 = psP.tile([65, JB], f32, tag="P", bufs=4,
                               name=f"pv0_{hp}")
                PV1 = psP.tile([65, JB], f32, tag="P", bufs=4,
                               name=f"pv1_{hp}")
                # software-pipelined: PV accumulation for chunk kc-1 issues
                # after the scores for chunk kc, so the in-order PE queue
                # never waits on the exp+mask of the chunk it just scored
                Es = []

                def pv_step(kc, PV0=PV0, PV1=PV1, h0=h0, h1=h1):
                    Ep = Es[kc]
                    nc.tensor.matmul(PV0[:], VA[kc][:, 65 * h0:65 * h0 + 65],
                                     Ep[:, 0:JB],
                                     start=(kc == 0), stop=(kc == KC - 1))
                    nc.tensor.matmul(PV1[:], VA[kc][:, 65 * h1:65 * h1 + 65],
                                     Ep[:, JB:2 * JB],
                                     start=(kc == 0), stop=(kc == KC - 1))

                for kc in range(KC):
                    SC = Stile(f"sc{hp}_{kc}")
                    nc.tensor.matmul(SC[:, 0:JB],
                                     KTs(h0)[:, 128 * kc:128 * (kc + 1)],
                                     QTs(h0), start=True, stop=True)
                    nc.tensor.matmul(SC[:, JB:2 * JB],
                                     KTs(h1)[:, 128 * kc:128 * (kc + 1)],
                                     QTs(h1), start=True, stop=True)
                    E = sp.tile([128, 1024], bf16, tag="e", bufs=4,
                                name=f"e{hp}_{kc}")
                    nc.scalar.activation(E[:], SC[:], Exp,
                                         scale=1.0 / np.sqrt(DH))
                    nc.vector.tensor_mul(E[:, 0:JB], E[:, 0:JB],
                                         maskb_sb[kc][:])
                    nc.vector.tensor_mul(E[:, JB:2 * JB], E[:, JB:2 * JB],
                                         maskb_sb[kc][:])
                    Es.append(E)
                    if kc > 0:
                        pv_step(kc - 1)
                pv_step(KC - 1)
                # normalize: copy PV to SBUF first so the PSUM slots free
                # right at stop (GPSIMD cannot touch PSUM, so DVE) and the
                # next head-pair's accumulation starts without waiting on
                # this pair's normalize chain. Row 64 is Z; R = 1/Z is
                # broadcast over 64 partitions via a ones-rank-1 matmul.
                PVs = sp.tile([65, 2 * JB], f32, tag="pvs", bufs=2,
                              name=f"pvs{hp}")
                nc.vector.tensor_copy(PVs[:, 0:JB], PV0[:])
                nc.vector.tensor_copy(PVs[:, JB:2 * JB], PV1[:])
                for half in (0, 1):
                    R = sp.tile([1, JB], f32r, tag="rb", bufs=1,
                                name=f"r{hp}_{half}")
                    with nc.allow_low_precision(
                            reason="f32r output is bit-identical to f32"):
                        nc.vector.reciprocal(
                            R[:], PVs[64:65, JB * half:JB * half + JB])
                    Rb = psP.tile([64, JB], f32, tag="P", bufs=4,
                                  name=f"rb{hp}_{half}")
                    nc.tensor.matmul(Rb[:], ones64_sb[:], R[:],
                                     start=True, stop=True)
                    # DVE may read only one PSUM operand: stage Rb in SBUF
                    Rs = sp.tile([64, JB], f32, tag="rb", bufs=1,
                                 name=f"rs{hp}_{half}")
                    nc.vector.tensor_copy(Rs[:], Rb[:])
                    nc.vector.tensor_mul(
                        FP[hp][64 * half:64 * half + 64, :],
                        PVs[0:64, JB * half:JB * half + JB], Rs[:])

            # ---------------- output projection ----------------
            wo_sb = []
            for ci in range(8):
                t = pp.tile([128, D], bf16, tag="wo", bufs=8,
                            name=f"wo{ci}")
                eng(ci + 1).dma_start(t[:], packed(R_WO + 64 * ci, 64, 1024))
                wo_sb.append(t)
            for p8 in range(8):
                ps = Stile(f"ops{p8}")
                for ci in range(8):
                    nc.tensor.matmul(ps[:, 0:JB],
                                     wo_sb[ci][:, 128 * p8:128 * (p8 + 1)],
                                     FP[ci][:], start=(ci == 0), stop=(ci == 7))
                osb = sp.tile([128, JB], bf16, tag="osb", bufs=3,
                              name=f"osb{p8}")
                nc.scalar.activation(osb[:], ps[:, 0:JB], Ident,
                                     bias=bo_col(p8))
                eng(p8).dma_start(out_d[128 * p8:128 * (p8 + 1), :], osb[:])

    nc.compile()
    return nc


def _get_module(S):
    if S not in _MODULES:
        _MODULES[S] = build_module(S)
    return _MODULES[S]


def host_shard(inputs, S):
    """Build the 8 per-core input maps from the full-size problem inputs."""
    q = np.asarray(inputs["queries"], dtype=np.float32)
    k = np.asarray(inputs["keys"], dtype=np.float32)
    v = np.asarray(inputs["values"], dtype=np.float32)
    mask = np.asarray(inputs["mask"])
    Wq = np.asarray(inputs["Wq"], dtype=np.float32)
    Wk = np.asarray(inputs["Wk"], dtype=np.float32)
    Wv = np.asarray(inputs["Wv"], dtype=np.float32)
    Wo = np.asarray(inputs["Wo"], dtype=np.float32)
    bq = np.asarray(inputs["bq"], dtype=np.float32)
    bk = np.asarray(inputs["bk"], dtype=np.float32)
    bv = np.asarray(inputs["bv"], dtype=np.float32)
    bo = np.asarray(inputs["bo"], dtype=np.float32)

    JB = S // 4
    JJ = S // 16
    # column order c = 128*i + jj  <->  source row s = 16*jj + i
    s_idx = (16 * np.arange(JJ)[None, :] + np.arange(16)[:, None]).reshape(-1)
    maskb_full = np.ascontiguousarray(
        (1 - mask[0, 0]).T.astype(ml_dtypes.bfloat16))
    qTs = [np.ascontiguousarray(q[b][s_idx].T.astype(ml_dtypes.bfloat16))
           for b in range(B)]
    kTs = [np.ascontiguousarray(k[b].T.astype(ml_dtypes.bfloat16))
           for b in range(B)]
    vTs = [np.ascontiguousarray(v[b].T.astype(ml_dtypes.bfloat16))
           for b in range(B)]
    wkT = Wk.T.astype(ml_dtypes.bfloat16)
    wvT = Wv.T.astype(ml_dtypes.bfloat16)
    woT = Wo.T.astype(ml_dtypes.bfloat16)
    bk_sb = bk.reshape(8, 128).T
    bo_sb = bo.reshape(8, 128).T
    bvrow_pad = np.zeros((1, 2048), dtype=ml_dtypes.bfloat16)
    bvrow_pad[0, :D] = bv.astype(ml_dtypes.bfloat16)

    in_maps = []
    for c in range(NCORES):
        b, g = divmod(c, 4)
        ch = slice(256 * g, 256 * g + 256)
        wq_slice = np.ascontiguousarray(
            Wq.T[:, ch].astype(ml_dtypes.bfloat16))
        biases = np.zeros((128, 32), dtype=ml_dtypes.bfloat16)
        biases[:, 0:2] = bq[ch].reshape(2, 128).T
        biases[:, 2:10] = bk_sb
        biases[:, 10:18] = bo_sb
        packb = np.concatenate([
            qTs[b].reshape(-1, 2048),
            kTs[b].reshape(-1, 2048),
            vTs[b].reshape(-1, 2048),
            np.ascontiguousarray(
                maskb_full[:, JB * g:JB * (g + 1)]).reshape(-1, 2048),
            wq_slice.reshape(-1, 2048),
            wkT.reshape(-1, 2048),
            wvT.reshape(-1, 2048),
            woT.reshape(-1, 2048),
            biases.reshape(-1, 2048),
            bvrow_pad,
        ], axis=0)
        in_maps.append({"packb": np.ascontiguousarray(packb)})
    return in_maps


def assemble(results, S):
    JB = S // 4
    out = np.empty((B, S, D), dtype=np.float32)
    for c in range(NCORES):
        b, g = divmod(c, 4)
        out[b, JB * g:JB * (g + 1), :] = \
            results[c]["ofinal"].T.astype(np.float32)
    return out


def kernel(**inputs):
    S = int(np.asarray(inputs["queries"]).shape[1])
    nc = _get_module(S)
    in_maps = host_shard(inputs, S)
    res = run_bass_kernel_spmd(nc, in_maps, core_ids=list(range(NCORES)))
    return assemble(res.results, S)
